# revision 2
# baseline (speedup 1.0000x reference)
"""DeformTransformerBlock2D Trainium2 kernel (8-core SPMD, full I/O).

Sharding: core k handles batch k//4, quarter k%4 of the 12800 positions
(3200 each). Each core computes the full-image value projection for its
batch (the bilinear gather is global).

Bilinear gather: all 64 (group, point) samples of a position lie in a
small window around the anchor cell. Window origin and size are placed
per-position on the host from an fp32 replica of the offset projection
(margin 0.125px vs the device's bf16 compute); positions are sorted by
required window size so most 128-position chunks need only ~4x4 taps.

The value field is stored bf16 in DRAM, row-major by pixel, with channels
interleaved as c' = ch*8 + g (group innermost). The per-group combined
tap weight cw[n,e,g] then broadcasts over ch on a middle AP dim while g
stays packed in the last dim, keeping every large TensorTensor in the
DVE 2x perf mode. Tap weight at window offset j is ReLU(1-|u-j|) (hat),
u = continuous in-window coordinate; hats vanish outside the window,
reproducing the reference's validity masking.
"""

import os
import numpy as np
import ml_dtypes

import concourse.bacc as bacc
import concourse.bass as bass
import concourse.tile as tile
from concourse import mybir
from concourse.bass_utils import run_bass_kernel_spmd

F32 = mybir.dt.float32
BF16 = mybir.dt.bfloat16
I16 = mybir.dt.int16
AX = mybir.AxisListType
ALU = mybir.AluOpType
ACTF = mybir.ActivationFunctionType

B, C, H, W = 2, 256, 80, 160
G, P_PTS = 8, 8
HW = H * W                     # 12800
NCORES = 8
NLOC = 3200                    # positions per core
NCH = 25                       # chunks of 128 positions
LN_EPS = 1e-5
MARGIN = 0.125
JMAX = 8                       # max window extent provisioned in jramp

_CACHE = {}
BF = ml_dtypes.bfloat16

# channel interleave: c' = ch*8 + g  <->  c = g*32 + ch
_CPERM = (np.arange(256).reshape(G, 32).T.reshape(-1))  # c'[j] -> original c


def _nsplit(total, step):
    o, out = 0, []
    while o < total:
        out.append((o, min(step, total - o)))
        o += step
    return out


def _build_program(shapes):
    nc = bacc.Bacc("TRN2", target_bir_lowering=False, debug=False,
                   num_devices=NCORES)
    gcols = sum(wy * 8 for wy, wx in shapes)

    d = {}
    def din(name, shape, dt):
        d[name] = nc.dram_tensor(name, shape, dt, kind="ExternalInput")
    din("xin_img", (2, 128, HW), BF16)
    din("xin_loc", (2, 128, NLOC), BF16)
    din("axy", (128, NCH * 2), F32)
    din("gidx", (128, gcols), I16)
    din("vWp", (256, 256), BF16)
    din("vbR", (1, 256), BF16)
    din("onesb", (1, 128), BF16)
    din("oaW", (256, 192), BF16)
    din("oabR", (128, 192), F32)      # host-replicated bias row
    din("outWp", (256, 256), BF16)
    din("outb", (128, 2), F32)
    din("w1T", (256, 512), BF16)
    din("b1", (128, 4), F32)
    din("w2T", (512, 256), BF16)
    din("b2", (128, 2), F32)
    din("ln1g", (128, 2), F32)
    din("ln1b", (128, 2), F32)
    din("ln2g", (128, 2), F32)
    din("ln2b", (128, 2), F32)
    din("jrampG", (128, JMAX * 64), BF16)
    din("ident", (128, 128), BF16)
    din("ones", (128, 1), BF16)       # column of ones (K=128 mean matmul)
    din("ones1", (1, 128), F32)       # row of ones (K=1 replication matmul)

    d["y_out"] = nc.dram_tensor("y_out", (2, 128, NLOC), F32,
                                kind="ExternalOutput")
    d["v8"] = nc.dram_tensor("v8scratch", (HW, 256), BF16)

    with tile.TileContext(nc) as tc:
        _emit(nc, tc, d, shapes)
    nc.compile()
    return nc


def _ld(nc, pool, dram, shape, dt, rearr=None, **rkw):
    t = pool.tile(shape, dt, tag="ld_" + dram.name)
    src = dram.ap()
    if rearr:
        src = src.rearrange(rearr, **rkw)
    nc.sync.dma_start(out=t, in_=src)
    return t


def _emit(nc, tc, d, shapes):
    from contextlib import ExitStack
    ctx = ExitStack()
    pconst = ctx.enter_context(tc.tile_pool(name="pconst", bufs=1))
    pmain = ctx.enter_context(tc.tile_pool(name="pmain", bufs=1))
    ppsA = ctx.enter_context(tc.tile_pool(name="ppsA", bufs=2, space="PSUM"))
    ppsT = ctx.enter_context(tc.tile_pool(name="ppsT", bufs=2, space="PSUM"))

    # ---------- constants ----------
    vWp = _ld(nc, pconst, d["vWp"], [128, 2, 256], BF16, "(kt k) m -> k kt m", k=128)
    vbR = _ld(nc, pconst, d["vbR"], [1, 256], BF16)
    onesb = _ld(nc, pconst, d["onesb"], [1, 128], BF16)
    oaW = _ld(nc, pconst, d["oaW"], [128, 2, 192], BF16, "(kt k) m -> k kt m", k=128)
    oabR = _ld(nc, pconst, d["oabR"], [128, 192], F32)
    outWp = _ld(nc, pconst, d["outWp"], [128, 2, 256], BF16, "(kt k) m -> k kt m", k=128)
    outb = _ld(nc, pconst, d["outb"], [128, 2], F32)
    w1T = _ld(nc, pconst, d["w1T"], [128, 2, 512], BF16, "(kt k) m -> k kt m", k=128)
    b1 = _ld(nc, pconst, d["b1"], [128, 4], F32)
    w2T = _ld(nc, pconst, d["w2T"], [128, 4, 256], BF16, "(kt k) m -> k kt m", k=128)
    b2 = _ld(nc, pconst, d["b2"], [128, 2], F32)
    ln1g = _ld(nc, pconst, d["ln1g"], [128, 2], F32)
    ln1b = _ld(nc, pconst, d["ln1b"], [128, 2], F32)
    ln2g = _ld(nc, pconst, d["ln2g"], [128, 2], F32)
    ln2b = _ld(nc, pconst, d["ln2b"], [128, 2], F32)
    axy = _ld(nc, pconst, d["axy"], [128, NCH * 2], F32)
    gcols = d["gidx"].shape[1]
    gidx = _ld(nc, pconst, d["gidx"], [128, gcols], I16)
    jrampG = _ld(nc, pconst, d["jrampG"], [128, JMAX, 64], BF16)
    ident = _ld(nc, pconst, d["ident"], [128, 128], BF16)
    ones = _ld(nc, pconst, d["ones"], [128, 1], BF16)
    ones1 = _ld(nc, pconst, d["ones1"], [1, 128], F32)

    # ---------- persistent activations ----------
    qb = pmain.tile([128, 2, NLOC], BF16)     # q + out_b (LN1 residual)
    aggT = pmain.tile([128, 2, NLOC], BF16)

    pmid_cm = tc.tile_pool(name="pmid", bufs=1)
    pmid = pmid_cm.__enter__()
    offa = pmid.tile([128, NCH, 192], F32)
    attnN = pmid.tile([128, NCH, G * P_PTS], BF16)

    # ========== phase 1: value field + projections ==========
    with tc.tile_pool(name="ph1", bufs=1) as p1, \
         tc.tile_pool(name="ph1t", bufs=3) as p1t, \
         tc.tile_pool(name="ppsB", bufs=2, space="PSUM") as ppsB:
        xl = _ld(nc, p1, d["xin_loc"], [128, 2, NLOC], BF16, "kt k n -> k kt n")
        xiap = d["xin_img"].ap().rearrange("kt k n -> k kt n")

        for kt in range(2):
            nc.vector.tensor_scalar_add(qb[:, kt], xl[:, kt],
                                        outb[:, kt:kt + 1])

        # value projection, X-stationary: psum[n, c'] = x[cin, n]^T vWp
        for pc in range(25):                   # 512-px chunks
            no = pc * 512
            xc = p1t.tile([128, 2, 512], BF16, tag="xc")
            nc.sync.dma_start(out=xc, in_=xiap[:, :, no:no + 512])
            vout = p1t.tile([128, 4, 256], BF16, tag="vout")
            for half in range(2):
                pv = ppsA.tile([128, 512], F32, tag="psA")
                pv2 = pv.rearrange("n (t m) -> n t m", t=2)
                for t in range(2):
                    sub = half * 2 + t
                    nc.tensor.matmul(pv2[:, t], onesb, vbR,
                                     start=True, stop=False)
                    for kt in range(2):
                        nc.tensor.matmul(
                            pv2[:, t],
                            xc[:, kt, sub * 128:(sub + 1) * 128],
                            vWp[:, kt, :], start=False, stop=(kt == 1))
                nc.scalar.activation(
                    vout[:, half * 2:(half + 1) * 2], pv2, ACTF.Copy)
            v8out = bass.AP(tensor=d["v8"], offset=no * 256,
                            ap=[[256, 128], [128 * 256, 4], [1, 256]])
            nc.sync.dma_start(out=v8out, in_=vout)

        # off/attn projections, chunk-stationary q
        for c in range(NCH):
            ps = ppsB.tile([128, 192], F32, tag="psB")
            for kt in range(2):
                nc.tensor.matmul(ps, xl[:, kt, c * 128:(c + 1) * 128],
                                 oaW[:, kt, :], start=(kt == 0), stop=(kt == 1))
            nc.vector.tensor_add(offa[:, c], ps, oabR)
            # softmax over points
            ae = p1t.tile([128, G, P_PTS], F32, tag="ae")
            nc.scalar.activation(ae.rearrange("n g p -> n (g p)"),
                                 offa[:, c, 128:192], ACTF.Exp)
            ssum = p1t.tile([128, G], F32, tag="ssum")
            nc.vector.tensor_reduce(ssum, ae, axis=AX.X, op=ALU.add)
            srec = p1t.tile([128, G], F32, tag="srec")
            nc.vector.reciprocal(srec, ssum)
            nc.vector.tensor_mul(
                attnN[:, c].rearrange("n (g p) -> n g p", g=G), ae,
                srec.unsqueeze(2).broadcast_to([128, G, P_PTS]))

    # ========== phase 2+3: gather + aggregation, interleaved LN/FFN ==========
    with tc.tile_pool(name="ph2w", bufs=2) as p2w, \
         tc.tile_pool(name="ph2m", bufs=2) as p2m, \
         tc.tile_pool(name="ph2t", bufs=4) as p2t, \
         tc.tile_pool(name="ph2s", bufs=2) as p2s, \
         tc.tile_pool(name="ph3t", bufs=1) as p3t, \
         tc.tile_pool(name="ppsM", bufs=2, space="PSUM") as ppsM:
        done_tiles = []
        def flush_tiles(upto):
            for no, nn in _nsplit(NLOC, 512):
                if no + nn <= upto and (no, nn) not in done_tiles:
                    done_tiles.append((no, nn))
                    _post_tile(nc, d, ppsA, ppsM, p3t, qb, aggT, outWp,
                               w1T, w2T, b1, b2, ln1g, ln1b, ln2g, ln2b,
                               ones, ones1, no, nn)
        goff = 0
        for c in range(NCH):
            WY, WX = shapes[c]
            E = WY * WX
            win = p2w.tile([128, WY, WX * 256], BF16, tag="win")
            v8in = bass.AP(tensor=d["v8"], offset=0,
                           ap=[[256, HW - WX + 1], [1, WX * 256]])
            nc.gpsimd.dma_gather(
                out_ap=win[:, :, :], in_ap=v8in,
                idxs_ap=gidx[:, goff:goff + WY * 8],
                num_idxs=WY * 128, num_idxs_reg=WY * 128,
                elem_size=WX * 256, elem_step=256)
            goff += WY * 8

            # hat weights: u = off + (anchor*S - 0.5 - origin), per axis
            u = p2t.tile([128, 2, 64], BF16, tag="u")
            offc = offa[:, c, 0:128].rearrange("n (gp two) -> n two gp", two=2)
            nc.vector.tensor_add(
                u, offc,
                axy[:, 2 * c:2 * c + 2].unsqueeze(2)
                   .broadcast_to([128, 2, 64]))
            lamc = p2t.tile([128, WY + WX, 64], BF16, tag="lamc")
            nc.vector.tensor_sub(
                lamc[:, 0:WY], u[:, 1:2].broadcast_to([128, WY, 64]),
                jrampG[:, 0:WY])
            nc.vector.tensor_sub(
                lamc[:, WY:WY + WX], u[:, 0:1].broadcast_to([128, WX, 64]),
                jrampG[:, 0:WX])
            lamf = lamc.rearrange("n j g -> n (j g)")
            nc.scalar.activation(lamf, lamf, ACTF.Abs)
            nc.scalar.activation(lamf, lamf, ACTF.Relu, bias=1.0, scale=-1.0)
            # fold attention into the y-hat
            cy = p2t.tile([128, WY, 64], BF16, tag="cy")
            nc.vector.tensor_mul(
                cy, lamc[:, 0:WY],
                attnN[:, c].unsqueeze(1).broadcast_to([128, WY, 64]))
            # outer product over (dy, dx), then reduce the 8 points
            prod = p2t.tile([128, WY, WX, 64], BF16, tag="prod")
            nc.vector.tensor_mul(
                prod, cy.unsqueeze(2).broadcast_to([128, WY, WX, 64]),
                lamc[:, WY:WY + WX].unsqueeze(1)
                    .broadcast_to([128, WY, WX, 64]))
            pe = prod.rearrange("n wy wx (g p) -> n (wy wx) g p", g=G)
            nc.vector.tensor_add(pe[:, :, :, 0:4], pe[:, :, :, 0:4],
                                 pe[:, :, :, 4:8])
            nc.vector.tensor_add(pe[:, :, :, 0:2], pe[:, :, :, 0:2],
                                 pe[:, :, :, 2:4])
            cw = p2s.tile([128, E, 8], BF16, tag="cw")
            nc.vector.tensor_add(cw, pe[:, :, :, 0], pe[:, :, :, 1])

            # FMA: tmp[n,e,ch,g] = win * cw (g packed, ch broadcast)
            winv = win.rearrange("n wy (wx c g) -> n (wy wx) c g",
                                 wx=WX, c=32)
            tmp = p2m.tile([128, E, 256], BF16, tag="tmp")
            nc.vector.tensor_mul(
                tmp.rearrange("n e (c g) -> n e c g", c=32), winv,
                cw.unsqueeze(2).broadcast_to([128, E, 32, 8]))
            rem = E
            while rem > 2:
                k = rem // 2
                nc.vector.tensor_add(tmp[:, :k], tmp[:, :k],
                                     tmp[:, rem - k:rem])
                rem -= k
            agb = p2m.tile([128, 256], BF16, tag="agb")
            nc.vector.tensor_add(agb, tmp[:, 0], tmp[:, 1])
            pst = ppsT.tile([128, 2, 128], BF16, tag="psT")
            for kt in range(2):
                nc.tensor.transpose(pst[:, kt], agb[:, kt * 128:(kt + 1) * 128],
                                    ident)
            nc.scalar.activation(aggT[:, :, c * 128:(c + 1) * 128], pst,
                                 ACTF.Copy)
            flush_tiles(c * 128)
        flush_tiles(NLOC)

    pmid_cm.__exit__(None, None, None)
    ctx.close()


def _ln_tile(nc, ppsA, ppsM, p3t, resid, xin, wT, lng, lnb, ones, ones1,
             yb_out, yf_out, no, nn, y_dram=None):
    """Per-512-tile: z = resid + wT.T @ xin; y = LN(z)*g+b (ch-major).
    resid/xin are tile-local views [128, kts, nn]."""
    kts = xin.shape[1]
    zt = p3t.tile([128, 2, 512], F32, tag="lnz")
    ztb = p3t.tile([128, 2, 512], BF16, tag="lnzb")
    for mt in range(2):
        ps = ppsA.tile([128, 512], F32, tag="psA")
        for kt in range(kts):
            nc.tensor.matmul(ps[:, :nn], wT[:, kt, mt * 128:(mt + 1) * 128],
                             xin[:, kt, :nn],
                             start=(kt == 0), stop=(kt == kts - 1))
        nc.vector.tensor_add(zt[:, mt, :nn], ps[:, :nn],
                             resid[:, mt, :nn])
        nc.scalar.copy(ztb[:, mt, :nn], zt[:, mt, :nn])
    psm = ppsM.tile([1, 512], F32, tag="psM")
    for kt in range(2):
        nc.tensor.matmul(psm[:1, :nn], ones, ztb[:, kt, :nn],
                         start=(kt == 0), stop=(kt == 1))
    sqt = p3t.tile([128, 2, 512], BF16, tag="lnsq")
    for mt in range(2):
        nc.scalar.activation(sqt[:, mt, :nn], zt[:, mt, :nn], ACTF.Square)
    psv = ppsM.tile([1, 512], F32, tag="psM")
    for kt in range(2):
        nc.tensor.matmul(psv[:1, :nn], ones, sqt[:, kt, :nn],
                         start=(kt == 0), stop=(kt == 1))
    mn = p3t.tile([1, 512], F32, tag="mn")
    nc.scalar.activation(mn[:, :nn], psm[:1, :nn], ACTF.Copy, scale=1.0 / 256)
    rs = p3t.tile([1, 512], F32, tag="rs")
    m2 = p3t.tile([1, 512], F32, tag="m2")
    nc.scalar.activation(m2[:, :nn], mn[:, :nn], ACTF.Square)
    nc.scalar.activation(rs[:, :nn], psv[:1, :nn], ACTF.Copy,
                         scale=1.0 / 256, bias=LN_EPS)
    nc.vector.tensor_sub(rs[:1, :nn], rs[:1, :nn], m2[:1, :nn])
    nc.scalar.activation(rs[:, :nn], rs[:, :nn], ACTF.Sqrt)
    nc.vector.reciprocal(rs[:1, :nn], rs[:1, :nn])
    nc.vector.tensor_mul(m2[:1, :nn], mn[:1, :nn], rs[:1, :nn])
    psr = ppsM.tile([128, 512], F32, tag="psR")
    nc.tensor.matmul(psr[:, :nn], ones1, rs[:1, :nn], start=True, stop=True)
    psr2 = ppsM.tile([128, 512], F32, tag="psR")
    nc.tensor.matmul(psr2[:, :nn], ones1, m2[:1, :nn], start=True, stop=True)
    for mt in range(2):
        nrm = p3t.tile([128, 512], F32, tag="nrm")
        nc.vector.tensor_mul(nrm[:, :nn], zt[:, mt, :nn], psr[:, :nn])
        nc.vector.tensor_sub(nrm[:, :nn], nrm[:, :nn], psr2[:, :nn])
        if y_dram is not None:
            yo = p3t.tile([128, 512], F32, tag="yo")
            nc.scalar.activation(yo[:, :nn], nrm[:, :nn], ACTF.Identity,
                                 scale=lng[:, mt:mt + 1], bias=lnb[:, mt:mt + 1])
            nc.sync.dma_start(out=y_dram[mt, :, no:no + nn], in_=yo[:, :nn])
        else:
            nc.scalar.activation(yf_out[:, mt, :nn], nrm[:, :nn],
                                 ACTF.Identity, scale=lng[:, mt:mt + 1],
                                 bias=lnb[:, mt:mt + 1])
            nc.scalar.copy(yb_out[:, mt, :nn], yf_out[:, mt, :nn])


def _post_tile(nc, d, ppsA, ppsM, p3t, qb, aggT, outWp, w1T, w2T, b1, b2,
               ln1g, ln1b, ln2g, ln2b, ones, ones1, no, nn):
    """out-proj + LN1 + FFN + LN2 + output DMA for positions [no, no+nn)."""
    y1f = p3t.tile([128, 2, 512], F32, tag="y1f")
    y1b = p3t.tile([128, 2, 512], BF16, tag="y1b")
    _ln_tile(nc, ppsA, ppsM, p3t, qb[:, :, no:no + nn],
             aggT[:, :, no:no + nn], outWp, ln1g, ln1b, ones, ones1,
             y1b, y1f, no, nn)
    hb = p3t.tile([128, 4, 512], BF16, tag="hb")
    use_silu = os.environ.get("KSIM", "0") != "1"
    for mt in range(4):
        ps = ppsA.tile([128, 512], F32, tag="psA")
        for kt in range(2):
            nc.tensor.matmul(ps[:, :nn], w1T[:, kt, mt * 128:(mt + 1) * 128],
                             y1b[:, kt, :nn], start=(kt == 0), stop=(kt == 1))
        if use_silu:
            nc.scalar.activation(hb[:, mt, :nn], ps[:, :nn], ACTF.Silu,
                                 bias=b1[:, mt:mt + 1])
        else:
            hx = p3t.tile([128, 512], F32, tag="hx")
            nc.scalar.activation(hx[:, :nn], ps[:, :nn], ACTF.Identity,
                                 bias=b1[:, mt:mt + 1])
            sg = p3t.tile([128, 512], F32, tag="sg")
            nc.scalar.activation(sg[:, :nn], ps[:, :nn], ACTF.Sigmoid,
                                 bias=b1[:, mt:mt + 1])
            nc.vector.tensor_mul(hb[:, mt, :nn], hx[:, :nn], sg[:, :nn])
    for kt in range(2):
        nc.scalar.activation(y1f[:, kt, :nn], y1f[:, kt, :nn], ACTF.Identity,
                             bias=b2[:, kt:kt + 1])
    _ln_tile(nc, ppsA, ppsM, p3t, y1f, hb, w2T, ln2g, ln2b, ones, ones1,
             None, None, no, nn, y_dram=d["y_out"])


def _plan(inputs):
    """Host analysis: per-core position permutation, window origins,
    shared per-chunk window shapes, gather indices."""
    f = np.asarray(inputs["feats"], np.float32)
    fp = np.asarray(inputs["feats_pos"], np.float32)
    anch = np.asarray(inputs["anchor_points"], np.float32)
    offW = np.asarray(inputs["off_W"], np.float32).astype(BF).astype(np.float32)
    offb = np.asarray(inputs["off_b"], np.float32)

    xb = [None, None]
    per_batch = []
    for b in range(B):
        x = (f[b].astype(BF).astype(np.float32)
             + fp[b].astype(BF).astype(np.float32)).reshape(C, HW)
        xb[b] = x.astype(BF)
        off = (xb[b].astype(np.float32).T @ offW + offb).reshape(HW, G, P_PTS, 2)
        ax = anch[b].reshape(HW, 2)[:, 0]
        ay = anch[b].reshape(HW, 2)[:, 1]
        px = ax[:, None] * W - 0.5 + off[..., 0].reshape(HW, -1)
        py = ay[:, None] * H - 0.5 + off[..., 1].reshape(HW, -1)
        txn = np.clip(np.floor(px.min(1) - MARGIN), 0, W - 1)
        txx = np.clip(np.floor(px.max(1) + MARGIN) + 1, 0, W - 1)
        tyn = np.clip(np.floor(py.min(1) - MARGIN), 0, H - 1)
        tyx = np.clip(np.floor(py.max(1) + MARGIN) + 1, 0, H - 1)
        per_batch.append((txn, txx - txn + 1, tyn, tyx - tyn + 1, ax, ay))

    cores = []
    wy_need = np.zeros((NCORES, NCH), np.int64)
    wx_need = np.zeros((NCORES, NCH), np.int64)
    for k in range(NCORES):
        b, s = k // 4, (k % 4) * NLOC
        txn, wx, tyn, wy = (a[s:s + NLOC] for a in per_batch[b][:4])
        order = np.argsort(wy * 16 + wx, kind="stable")
        cores.append((b, s, order))
        wys, wxs = wy[order], wx[order]
        for c in range(NCH):
            wy_need[k, c] = wys[c * 128:(c + 1) * 128].max()
            wx_need[k, c] = wxs[c * 128:(c + 1) * 128].max()
    shapes = tuple((int(wy_need[:, c].max()), int(wx_need[:, c].max()))
                   for c in range(NCH))
    return xb, per_batch, cores, shapes


def _prep_inputs(inputs, xb, per_batch, cores, shapes):
    offW = np.asarray(inputs["off_W"], np.float32)
    attnW = np.asarray(inputs["attn_W"], np.float32)
    oab = np.concatenate([np.asarray(inputs["off_b"], np.float32),
                          np.asarray(inputs["attn_b"], np.float32)])

    def bf(x):
        return np.asarray(x, np.float32).astype(BF)

    jr = np.zeros((JMAX, 64), np.float32)
    jr[:] = np.arange(JMAX, dtype=np.float32)[:, None]
    shared = {
        "vWp": bf(np.asarray(inputs["value_W"], np.float32)[:, _CPERM]),
        "vbR": bf(np.asarray(inputs["value_b"], np.float32)[_CPERM])
            .reshape(1, 256),
        "onesb": np.ones((1, 128), np.float32).astype(BF),
        "oaW": bf(np.concatenate([offW, attnW], axis=1)),
        "oabR": np.ascontiguousarray(np.broadcast_to(oab, (128, 192))),
        "outWp": bf(np.asarray(inputs["out_W"], np.float32)[_CPERM, :]),
        "outb": np.ascontiguousarray(
            np.asarray(inputs["out_b"], np.float32).reshape(2, 128).T),
        "w1T": bf(np.asarray(inputs["ffn_w1"], np.float32).T),
        "b1": np.ascontiguousarray(
            np.asarray(inputs["ffn_b1"], np.float32).reshape(4, 128).T),
        "w2T": bf(np.asarray(inputs["ffn_w2"], np.float32).T),
        "b2": np.ascontiguousarray(
            np.asarray(inputs["ffn_b2"], np.float32).reshape(2, 128).T),
        "ln1g": np.ascontiguousarray(
            np.asarray(inputs["ln1_g"], np.float32).reshape(2, 128).T),
        "ln1b": np.ascontiguousarray(
            np.asarray(inputs["ln1_b"], np.float32).reshape(2, 128).T),
        "ln2g": np.ascontiguousarray(
            np.asarray(inputs["ln2_g"], np.float32).reshape(2, 128).T),
        "ln2b": np.ascontiguousarray(
            np.asarray(inputs["ln2_b"], np.float32).reshape(2, 128).T),
        "jrampG": np.ascontiguousarray(
            np.broadcast_to(jr.reshape(1, -1), (128, JMAX * 64))).astype(BF),
        "ident": np.eye(128, dtype=np.float32).astype(BF),
        "ones": np.ones((128, 1), np.float32).astype(BF),
        "ones1": np.ones((1, 128), np.float32),
    }

    in_maps = []
    for k in range(NCORES):
        b, s, order = cores[k]
        txn, wx, tyn, wy, ax, ay = per_batch[b]
        idx = s + order                      # original position ids
        oxs = np.empty(NLOC, np.int64)
        oys = np.empty(NLOC, np.int64)
        for c in range(NCH):
            WY, WX = shapes[c]
            sl = idx[c * 128:(c + 1) * 128]
            oxs[c * 128:(c + 1) * 128] = np.minimum(txn[sl], W - WX)
            oys[c * 128:(c + 1) * 128] = np.minimum(tyn[sl], H - WY)
        axm = (ax[idx] * W - 0.5 - oxs).astype(np.float32)
        aym = (ay[idx] * H - 0.5 - oys).astype(np.float32)
        m0 = oys * W + oxs

        cols = []
        for c in range(NCH):
            WY, WX = shapes[c]
            m0c = m0[c * 128:(c + 1) * 128]
            g16 = np.zeros((16, WY * 8), np.int64)
            for dy in range(WY):
                v = (m0c + dy * W).reshape(8, 16)
                g16[:, dy * 8:(dy + 1) * 8] = v.T
            cols.append(g16)
        gidx = np.tile(np.concatenate(cols, axis=1), (8, 1)).astype(np.int16)

        m = dict(shared)
        m["xin_img"] = np.ascontiguousarray(xb[b].reshape(2, 128, HW))
        m["xin_loc"] = np.ascontiguousarray(
            xb[b][:, idx].reshape(2, 128, NLOC))
        axm2 = np.ascontiguousarray(axm.reshape(NCH, 128).T)
        aym2 = np.ascontiguousarray(aym.reshape(NCH, 128).T)
        axy = np.stack([axm2, aym2], axis=2)   # [128, NCH, 2]
        m["axy"] = np.ascontiguousarray(axy.reshape(128, NCH * 2))
        m["gidx"] = gidx
        in_maps.append(m)
    return in_maps


def kernel(**inputs):
    xb, per_batch, cores, shapes = _plan(inputs)
    if _CACHE.get("shapes") != shapes:
        _CACHE["nc"] = _build_program(shapes)
        _CACHE["shapes"] = shapes
    nc = _CACHE["nc"]
    in_maps = _prep_inputs(inputs, xb, per_batch, cores, shapes)
    trace = bool(int(os.environ.get("KTRACE", "0")))
    res = run_bass_kernel_spmd(nc, in_maps, core_ids=list(range(NCORES)),
                               trace=trace)
    _CACHE["exec_time_ns"] = res.exec_time_ns
    _CACHE["trace"] = res.instructions_and_trace
    out = np.zeros((B, C, HW), np.float32)
    for k in range(NCORES):
        b, s, order = cores[k]
        out[b][:, s + order] = res.results[k]["y_out"].reshape(C, NLOC)
    return out.reshape(B, C, H, W)
